# revision 2
# baseline (speedup 1.0000x reference)
"""Trainium2 Bass kernel for nn_AMM_w_AFDM (scatter_memory).

Strategy (one batch per NeuronCore, 8 cores data-parallel):
  out[b] = feature + P + splat(P, w)  where P = nearest-cell scatter of x.
  The 11x11 splat-with-border-clipping is a bank of banded Toeplitz
  matmuls on TensorE (fp8 DoubleRow); P is built with
  gpsimd.dma_scatter_add (SBUF parity-split CCE-add destination).  The
  exact-dedup (HW races on duplicate indices within a call) runs in
  4-chunk groups pipelined with the scatter calls, with the lin
  broadcast done by stride-0 one-hot matmuls on a transposed floor
  chain (no DRAM roundtrip).  The AFDM conv1x1 runs in 2-PSUM-bank
  blocks during the dedup phase; the dilation scalar chain, the w
  kernel, and the P merges overlap the scatter window and the conv.
  All constants ship pre-cast; the whole kernel uses one ACT table.
"""
import sys
from contextlib import ExitStack
import numpy as np

sys.path.insert(0, "/opt/trn_rl_repo")

import concourse.bacc as bacc  # noqa: E402
import concourse.bass as bass  # noqa: E402
import concourse.mybir as mybir  # noqa: E402
import concourse.tile as tile  # noqa: E402
from concourse.ap import AP  # noqa: E402

MD = 5
EPS = 1e-5
B, N, C, H, W = 8, 4096, 64, 128, 128
F32 = mybir.dt.float32
BF16 = mybir.dt.bfloat16
I16 = mybir.dt.int16
AX = mybir.AxisListType
OP = mybir.AluOpType
AF = mybir.ActivationFunctionType

# P_T slot layout: slot t in [0,160), y = t-16 (zeros outside [0,128)).
PT_SLOTS = 160
PT_OFF = 16
# ext output slots: u in [0,144), ye = u-8.
EXT_SLOTS = 144


def build_nc():
    nc = bacc.Bacc("TRN2", target_bir_lowering=False)

    def din(name, shape, dt=F32):
        return nc.dram_tensor(name, shape, dt, kind="ExternalInput")

    x_wrap = din("x_wrap", [128, 32, 64], BF16)
    xxy_w2 = din("xxy_w2", [128, 64])
    feat_nat2 = din("feat_nat2", [128, 8192], BF16)
    feat_T = din("feat_T", [128, 8192], BF16)
    blob1 = din("blob1", [128, 968])
    blob2 = din("blob2", [11, 547])
    sblob = din("sblob", [128, 1858], BF16)
    xt_blob = din("xt_blob", [32, 288])
    w1n = din("w1n", [128, 32, 128], BF16)

    out_T = nc.dram_tensor("out_T", [128, 8192], BF16, kind="ExternalOutput")
    vdram = nc.dram_tensor("vdram", [11, 400], F32)

    with tile.TileContext(nc) as tc:
        with tc.tile_pool(name="main", bufs=1) as pool, \
             tc.tile_pool(name="scat", bufs=1) as scpool, \
             tc.tile_pool(name="stage", bufs=2) as stpool:
            psum_stack = ExitStack()

            # ============== early input DMAs ==============
            ps_prep = psum_stack.enter_context(
                tc.tile_pool(name="psp", bufs=2, space="PSUM"))
            xt_sb = pool.tile([32, 288], F32, tag="xt_sb")
            nc.sync.dma_start(out=xt_sb[:, :], in_=xt_blob[:, :])
            xxy2 = pool.tile([128, 64], F32, tag="xxy2")
            nc.sync.dma_start(out=xxy2[:, :], in_=xxy_w2[:, :])
            sblob_sb = pool.tile([128, 1858], BF16, tag="sblob_sb")
            nc.sync.dma_start(out=sblob_sb[:, :], in_=sblob[:, :])
            blob1_sb = pool.tile([128, 968], F32, tag="blob1_sb")
            nc.sync.dma_start(out=blob1_sb[:, :], in_=blob1[:, :])
            blob2_sb = pool.tile([11, 547], F32, tag="blob2_sb")
            nc.sync.dma_start(out=blob2_sb[:, :], in_=blob2[:, :])
            x_bf = scpool.tile([128, 32, 64], BF16, tag="x_bf")
            nc.sync.dma_start(out=x_bf[:, :, :], in_=x_wrap[:, :, :])
            fnat_bf = pool.tile([128, 8192], BF16, tag="fnat_bf")
            nc.sync.dma_start(out=fnat_bf[:, :], in_=feat_nat2[:, :])
            w1_bf = pool.tile([128, 32, 128], BF16, tag="w1_bf")

            # ============== Phase S: scatter x -> P ==============
            # Index math in the 128-wrap: lin128[p, t] for point j = t*128+p.
            # (emitted FIRST on the DVE queue so the dedup pipeline starts
            #  immediately; buffer memsets follow on gpsimd/DVE)
            # floor(t) = round(t) - (round(t) > t), round via +/- 2^23;
            # x and y coords packed in one [128,64] tile to halve the serial
            # DVE chain (each small DVE op costs ~1us of fixed overhead)
            # floor(t) = round(t - 0.5) via the 2^23 trick fused into one
            # 2-op tensor_scalar; two layouts: [32,*] (transposed, feeds the
            # dedup broadcast matmuls directly - no PE transpose / DMA fold)
            # and [128,*] (feeds eq-compare scalars and the idx pack).
            # magic constant 1.5*2^23 keeps the sum in the ulp=1 region for
            # all t-0.5 >= -0.5 (plain 2^23 breaks for t<0.25: the sum lands
            # in the 0.5-ulp band just below 2^23).  xyz is pre-scaled by 127
            # host-side so the 2-op tensor_scalar can spend both ops on adds.
            RC = 12582912.0
            rT = pool.tile([32, 256], F32, tag="rT")
            nc.vector.tensor_scalar(rT[:, :], xt_sb[:, 0:256], -0.5,
                                    RC, OP.add, OP.add)
            flT = pool.tile([32, 256], F32, tag="flT")
            nc.vector.tensor_scalar_add(flT[:, :], rT[:, :], -RC)
            linT = pool.tile([32, 128], F32, tag="linT")
            nc.vector.scalar_tensor_tensor(linT[:, :], flT[:, 128:256], 128.0,
                                           flT[:, 0:128], OP.mult, OP.add)
            r128 = pool.tile([128, 64], F32, tag="r128")
            nc.vector.tensor_scalar(r128[:, :], xxy2[:, :], -0.5,
                                    RC, OP.add, OP.add)
            fl = pool.tile([128, 64], F32, tag="fl")
            nc.vector.tensor_scalar_add(fl[:, :], r128[:, :], -RC)
            lin128 = pool.tile([128, 32], F32, tag="lin128")
            nc.vector.scalar_tensor_tensor(lin128[:, :], fl[:, 32:64], 128.0,
                                           fl[:, 0:32], OP.mult, OP.add)

            pcol_col = blob1_sb[:, 256:257]
            th_f = blob1_sb[:, 257:289]
            gm_v = blob1_sb[:, 960:964]
            cb_col = blob1_sb[:, 964:965]
            gg_col = blob1_sb[:, 965:966]
            gb_col = blob1_sb[:, 966:967]
            onesc_f = blob1_sb[:, 967:968]
            gmt_v = blob2_sb[0:4, 0:128]
            gr8_v = blob2_sb[0:8, 128:256]
            w2_v = blob2_sb[0:1, 256:384]
            o1128_v = blob2_sb[0:1, 384:512]
            dnt_v = blob2_sb[0:11, 512:523]
            cm_v = blob2_sb[0:11, 523:534]
            o11_v = blob2_sb[0:11, 534:535]
            o111_v = blob2_sb[0:1, 535:546]
            # bf16 constants shipped pre-cast in sblob (DVE casts of consts
            # were costing ~3us of serial head latency each)
            sba = sblob_sb[:, :]
            _sp = list(sba.ap[0])

            def sb_ap(off, dims):
                return AP(tensor=sba.tensor, offset=sba.offset + off,
                          ap=[_sp] + dims)

            def sel8h_ap(gg):
                return sb_ap(gg * 128, [[1, 128]])

            th_bf_all = sb_ap(1024, [[1, 32]])
            ltri2_ap = sb_ap(1056, [[0, 2], [1, 128]])
            onesc_bf = sb_ap(1184, [[1, 1]])
            ident_bf = sb_ap(1185, [[1, 128]])

            xta = xt_sb[:, :]

            def e32_ap(t0):
                return AP(tensor=xta.tensor, offset=xta.offset + 256 + t0,
                          ap=[list(xta.ap[0]), [0, 128]])

            # post-floor helpers for the idx pack (off the critical chain)
            xcq = pool.tile([128, 32], F32, tag="xcq")
            nc.vector.tensor_scalar(xcq[:, :], fl[:, 0:32], pcol_col,
                                    None, OP.subtract)
            ytr = pool.tile([128, 32], F32, tag="ytr")
            nc.vector.tensor_tensor(ytr[:, :], fl[:, 32:64], th_f, OP.subtract)
            nc.vector.tensor_scalar_mul(ytr[:, :], ytr[:, :], 128.0)

            # scatter destination buffers: memsets on gpsimd, which is idle
            # until the first scatter call anyway — keeps DVE free for dedup
            pb0 = scpool.tile([128, 66 * 64], BF16, tag="pb0")
            pb1 = scpool.tile([128, 66 * 64], BF16, tag="pb1")
            pb2 = scpool.tile([128, 66 * 64], BF16, tag="pb2")
            pb3 = scpool.tile([128, 66 * 64], BF16, tag="pb3")
            pbuf = [pb0, pb1, pb2, pb3]  # [A_even, A_odd, B_even, B_odd]
            # dummy scatter first on the gpsimd queue: forces the Q7 'mlp'
            # library DMA+boot (~10us) at t~4 instead of right before the
            # first real scatter call
            dummy_v = pool.tile([128, 64], BF16, tag="dummy_v")
            nc.vector.memset(dummy_v[:, :], 0.0)
            dummy_i = pool.tile([128, 8], I16, tag="dummy_i")
            nc.vector.memset(dummy_i[:, :], 0)
            # target the never-read trash columns of pb0/pb1: harmless, and
            # the WAW hazard forces the scheduler to run this (and the lib
            # boot it triggers) BEFORE the memsets instead of reordering it
            nc.gpsimd.dma_scatter_add(
                pb0[:, 4096:4160],
                dummy_v[:, :].rearrange("p (a b) -> p a b", b=64),
                dummy_i[:, :], 128, 128, 64,
                parity_reg=0, out_ap_other=pb1[:, 4096:4160],
                sbuf_tokens_per_rank=128)
            for pb in pbuf:
                nc.gpsimd.memset(pb[:, 0:4096], 0.0)
            # P accumulators (bf16 for the exact identity term, fp8 for the
            # DoubleRow Toeplitz conv); border memsets emitted after dedup
            p_t = pool.tile([128, PT_SLOTS * 64], BF16, tag="p_t")
            p8 = pool.tile([128, PT_SLOTS * 64], mybir.dt.float8e4, tag="p8")

            # Per-256-chunk dedup, pipelined in 4-chunk GROUPS so the first
            # scatter call issues as soon as chunks 0-3 are deduped (instead
            # of after a full 8-chunk half).  chunk a = subs (u,v)=(2a,2a+1).
            mqt = [pool.tile([128, 512], BF16, tag=f"mq{i}", name=f"mq{i}")
                   for i in range(2)]
            mlt = [pool.tile([128, 256], BF16, tag=f"ml{i}", name=f"ml{i}")
                   for i in range(2)]
            xmg = [scpool.tile([128, 8, 64], BF16, tag=f"xmg{i}",
                               name=f"xmg{i}") for i in range(4)]
            idxg = [scpool.tile([128, 64], I16, tag=f"idxg{i}",
                                name=f"idxg{i}") for i in range(4)]
            hilot = [pool.tile([128, 16], BF16, tag=f"hilo{i}",
                               name=f"hilo{i}") for i in range(4)]
            psq = psum_stack.enter_context(
                tc.tile_pool(name="psq", bufs=2, space="PSUM"))
            psg = psum_stack.enter_context(
                tc.tile_pool(name="psg", bufs=2, space="PSUM"))
            psA = psum_stack.enter_context(
                tc.tile_pool(name="psA", bufs=2, space="PSUM"))
            r_sb = pool.tile([128, 2048], F32, tag="r_sb")

            for g in range(4):
                hs = slice(8 * g, 8 * g + 8)
                pm = psg.tile([128, 512], F32, tag="pm")
                aux = psg.tile([128, 80], F32, tag="aux")
                cntp = aux[:, 0:8]
                psw = aux[:, 16:80]
                firstp = stpool.tile([128, 8], F32, tag="firstp")
                firstp = firstp[:, :]
                for c in range(4):
                    a = 4 * g + c
                    u, v = 2 * a, 2 * a + 1
                    # bc[q, 0:128|128:256] = lin(p, u|v) bcast across q, via
                    # stride-0 one-hot lhsT on the transposed lin (K=32)
                    bc = psq.tile([128, 256], F32, tag="bc")
                    nc.tensor.matmul(bc[:, 0:128], e32_ap(u), linT[:, :],
                                     start=True, stop=True)
                    nc.tensor.matmul(bc[:, 128:256], e32_ap(v), linT[:, :],
                                     start=True, stop=True)
                    # eq tiles packed as mq = [m_uu | m_uv | m_vu | m_vv]
                    mq = mqt[a % 2]
                    nc.vector.tensor_scalar(mq[:, 0:256], bc[:, 0:256],
                                            lin128[:, u:u + 1], None,
                                            OP.is_equal)
                    nc.vector.tensor_scalar(mq[:, 256:512], bc[:, 0:256],
                                            lin128[:, v:v + 1], None,
                                            OP.is_equal)
                    # ml = [m_uu*L | m_vv*L] in ONE strided op
                    ml = mlt[a % 2]
                    mqa = mq[:, :]
                    nc.vector.tensor_tensor(
                        ml[:, :].rearrange("p (a b) -> p a b", b=128),
                        AP(tensor=mqa.tensor, offset=mqa.offset,
                           ap=[list(mqa.ap[0]), [384, 2], [1, 128]]),
                        ltri2_ap, OP.mult)
                    # merged values
                    nc.tensor.matmul(pm[:, c * 128:c * 128 + 64],
                                     mq[:, 0:128], x_bf[:, u, :],
                                     start=True, stop=False)
                    nc.tensor.matmul(pm[:, c * 128:c * 128 + 64],
                                     mq[:, 256:384], x_bf[:, v, :],
                                     start=False, stop=True)
                    nc.tensor.matmul(pm[:, c * 128 + 64:c * 128 + 128],
                                     mq[:, 384:512], x_bf[:, v, :],
                                     start=True, stop=True)
                    # duplicate-below counts
                    nc.tensor.matmul(cntp[:, 2 * c:2 * c + 1], ml[:, 0:128],
                                     onesc_bf, start=True, stop=True)
                    nc.tensor.matmul(cntp[:, 2 * c + 1:2 * c + 2],
                                     ml[:, 128:256], onesc_bf,
                                     start=True, stop=False)
                    nc.tensor.matmul(cntp[:, 2 * c + 1:2 * c + 2],
                                     mq[:, 128:256], onesc_bf,
                                     start=False, stop=True)
                nc.vector.tensor_scalar(firstp, cntp, 0.5,
                                        None, OP.is_lt)
                nc.scalar.activation(xmg[g][:, :, :], pm[:, :], AF.Copy)
                # idx split into bf16-exact hi=y / lo=x halves (trash cells
                # when not-first), wrapped+replicated by one-hot bf16 matmuls
                hilo = hilot[g]
                nc.vector.tensor_tensor(hilo[:, 0:8], ytr[:, hs],
                                        firstp, OP.mult)
                nc.vector.tensor_tensor(
                    hilo[:, 0:8], hilo[:, 0:8],
                    AP(tensor=sba.tensor, offset=sba.offset + 1024 + 8 * g,
                       ap=[_sp, [1, 8]]), OP.add)
                nc.vector.tensor_tensor(hilo[:, 8:16], xcq[:, hs],
                                        firstp, OP.mult)
                nc.vector.tensor_tensor(hilo[:, 8:16], hilo[:, 8:16],
                                        sb_ap(1857, [[0, 8]]), OP.add)
                for gg in range(8):
                    pw_ap = AP(tensor=psw.tensor, offset=psw.offset + gg * 8,
                               ap=[list(psw.ap[0]), [1, 8]])
                    nc.tensor.matmul(pw_ap, sel8h_ap(gg), hilo[:, 8:16],
                                     start=True, stop=False)
                    nc.tensor.matmul(pw_ap, sel8h_ap(gg), hilo[:, 0:8],
                                     start=False, stop=True)
                ida = idxg[g][:, :]
                nc.vector.tensor_copy(
                    AP(tensor=ida.tensor, offset=ida.offset,
                       ap=[list(ida.ap[0]), [16, 4], [8, 2], [1, 8]]),
                    AP(tensor=psw.tensor, offset=psw.offset,
                       ap=[list(psw.ap[0]), [2, 4], [1, 2], [8, 8]]))
                for c in range(4):
                    a = 4 * g + c
                    ch = a % 2
                    nc.gpsimd.dma_scatter_add(
                        pbuf[2 * ch][:, :], xmg[g][:, 2 * c:2 * c + 2, :],
                        idxg[g][:, c * 16:(c + 1) * 16],
                        256, 256, 64,
                        parity_reg=0, out_ap_other=pbuf[2 * ch + 1][:, :],
                        sbuf_tokens_per_rank=128,
                    )

            # AFDM conv1x1 in 2-bank blocks DURING the dedup/scatter phase
            # (fits beside the dedup psum pools), spilled to r_sb so the GN
            # stats chain starts ~20us earlier
            for blk in range(4):
                psr = psA.tile([128, 512], F32, tag="psr")
                for g in range(4):
                    nc.tensor.matmul(psr[:, :],
                                     sb_ap(1313 + 128 * g, [[1, 128]]),
                                     fnat_bf[:, (g * 4 + blk) * 512:
                                             (g * 4 + blk) * 512 + 512],
                                     start=(g == 0), stop=(g == 3))
                nc.scalar.activation(r_sb[:, blk * 512:(blk + 1) * 512],
                                     psr[:, :], AF.Copy)

            # big input DMAs deferred here: they land during the scatter
            # window instead of clogging SBUF write ports during the dedup
            # index math at the head
            nc.sync.dma_start(out=w1_bf[:, :, :], in_=w1n[:, :, :])
            ft = pool.tile([128, 8192], BF16, tag="ft")
            nc.sync.dma_start(out=ft[:, :], in_=feat_T[:, :])

            # P border slots (small; DVE reaches these right after dedup)
            nc.vector.memset(p_t[:, 0:PT_OFF * 64], 0.0)
            nc.vector.memset(p_t[:, (PT_OFF + 128) * 64:], 0.0)
            nc.vector.memset(p8[:, 0:PT_OFF * 64], 0.0)
            nc.vector.memset(p8[:, (PT_OFF + 128) * 64:], 0.0)

            psum_stack.close()
            psum_stack = ExitStack()
            psa = psum_stack.enter_context(
                tc.tile_pool(name="psa", bufs=1, space="PSUM"))

            # ============== Phase A: AFDM dilation ==============
            # (emitted before the P merge so its DVE/ACT/PE ops run during the
            #  scatter window)
            # sf sums (conv_b is zeros per the problem spec, so r_sb needs
            # no bias pass; it was spilled block-wise during the dedup phase)
            psum_sf = psa.tile([8, 2048], F32, tag="psum_sf")
            for wnd in range(16):
                g = wnd // 4
                st_flag = wnd < 4
                sp_flag = wnd >= 12
                rhs = fnat_bf[:, wnd * 512:(wnd + 1) * 512]
                nc.tensor.matmul(psum_sf[:, (wnd % 4) * 512:(wnd % 4) * 512 + 512],
                                 sb_ap(1825 + 8 * g, [[1, 8]]), rhs,
                                 start=st_flag, stop=sp_flag)
            # sf: min/max + pool
            sfmm = pool.tile([8, 2], F32, tag="sfmm")
            nc.vector.tensor_reduce(sfmm[:, 0:1], psum_sf[:, :], AX.X, OP.min)
            nc.vector.tensor_reduce(sfmm[:, 1:2], psum_sf[:, :], AX.X, OP.max)
            pool1sf = pool.tile([8, 256], F32, tag="pool1sf")
            nc.vector.tensor_reduce(pool1sf[:, :],
                                    psum_sf[:, :].rearrange("p (a b) -> p a b", b=8),
                                    AX.X, OP.add)
            pool2sf = pool.tile([8, 32], F32, tag="pool2sf")
            p1s = pool1sf[:, :]
            nc.vector.tensor_reduce(
                pool2sf[:, :].rearrange("p (a b) -> p a b", b=16),
                AP(tensor=p1s.tensor, offset=p1s.offset,
                   ap=[list(p1s.ap[0]), [128, 2], [1, 16], [16, 8]]),
                AX.X, OP.add)

            # close psa (psum_r / psum_sf fully consumed), open psb
            psum_stack.close()
            psum_stack = ExitStack()
            psb = psum_stack.enter_context(
                tc.tile_pool(name="psb", bufs=1, space="PSUM"))
            # stats: s1 = sum r, s2 = sum r^2 (ACT Square w/ accum)
            s_col = pool.tile([128, 2], F32, tag="s_col")
            nc.vector.tensor_reduce(s_col[:, 0:1], r_sb[:, :], AX.X, OP.add)
            rr_sb = pool.tile([128, 2048], BF16, tag="rr_sb")
            nc.scalar.activation(rr_sb[:, :], r_sb[:, :], AF.Square,
                                 accum_out=s_col[:, 1:2])
            psum_st = psb.tile([4, 2], F32, tag="psum_st")
            nc.tensor.matmul(psum_st[:, :], gm_v, s_col[:, :],
                             start=True, stop=True)
            # mu, rstd
            stt = pool.tile([4, 6], F32, tag="stt")
            inv_cnt = 1.0 / (4 * H * W)
            nc.vector.tensor_scalar_mul(stt[:, 0:1], psum_st[:, 0:1], inv_cnt)
            nc.vector.tensor_tensor(stt[:, 2:3], stt[:, 0:1], stt[:, 0:1], OP.mult)
            nc.vector.scalar_tensor_tensor(stt[:, 3:4], psum_st[:, 1:2],
                                           inv_cnt, stt[:, 2:3],
                                           OP.mult, OP.subtract)
            # rstd = exp(-0.5*ln(var+eps)): keeps every ACT op in the single
            # natural_log_exp_and_others table (no mid-kernel table swaps)
            nc.scalar.activation(stt[:, 4:5], stt[:, 3:4], AF.Ln,
                                 bias=blob2_sb[0:4, 546:547])
            nc.scalar.activation(stt[:, 1:2], stt[:, 4:5], AF.Exp, scale=-0.5)
            sta = stt[:, :]
            psum_bc = psb.tile([128, 2], F32, tag="psum_bc")
            nc.tensor.matmul(psum_bc[:, :], gmt_v,
                             AP(tensor=sta.tensor, offset=sta.offset,
                                ap=[list(sta.ap[0]), [1, 2]]),
                             start=True, stop=True)
            a_col = pool.tile([128, 1], F32, tag="a_col")
            b_col = pool.tile([128, 1], F32, tag="b_col")
            nc.vector.tensor_tensor(a_col[:, :], psum_bc[:, 1:2], gg_col[:, :], OP.mult)
            nc.vector.tensor_tensor(b_col[:, :], psum_bc[:, 0:1], a_col[:, :], OP.mult)
            nc.vector.tensor_tensor(b_col[:, :], gb_col[:, :], b_col[:, :], OP.subtract)
            # rr = relu(r*a + b)
            nc.scalar.activation(rr_sb[:, :], r_sb[:, :], AF.Relu,
                                 bias=b_col[:, 0:1], scale=a_col[:, 0:1])
            # pool 8x8
            pool1 = pool.tile([128, 256], F32, tag="pool1")
            nc.vector.tensor_reduce(pool1[:, :],
                                    rr_sb[:, :].rearrange("p (a b) -> p a b", b=8),
                                    AX.X, OP.add)
            pool2 = pool.tile([128, 32], F32, tag="pool2")
            p1a = pool1[:, :]
            nc.vector.tensor_reduce(
                pool2[:, :].rearrange("p (a b) -> p a b", b=16),
                AP(tensor=p1a.tensor, offset=p1a.offset,
                   ap=[list(p1a.ap[0]), [128, 2], [1, 16], [16, 8]]),
                AX.X, OP.add)

            mm2t = pool.tile([1, 16], F32, tag="mm2t")
            nc.sync.dma_start(out=mm2t[:, :], in_=sfmm[:, :])
            gmn = pool.tile([1, 4], F32, tag="gmn")
            mma = mm2t[:, :]
            nc.vector.tensor_reduce(
                gmn[:, 0:1],
                AP(tensor=mma.tensor, offset=mma.offset, ap=[list(mma.ap[0]), [2, 8]]),
                AX.X, OP.min)
            nc.vector.tensor_reduce(
                gmn[:, 1:2],
                AP(tensor=mma.tensor, offset=mma.offset + 1,
                   ap=[list(mma.ap[0]), [2, 8]]),
                AX.X, OP.max)
            nc.vector.tensor_tensor(gmn[:, 2:3], gmn[:, 1:2], gmn[:, 0:1], OP.subtract)
            nc.vector.tensor_scalar_add(gmn[:, 2:3], gmn[:, 2:3], EPS)
            nc.vector.reciprocal(gmn[:, 3:4], gmn[:, 2:3])
            pack12 = pool.tile([1, 2], F32, tag="pack12")
            nc.vector.tensor_copy(pack12[:, 0:1], gmn[:, 0:1])
            nc.vector.tensor_copy(pack12[:, 1:2], gmn[:, 3:4])
            psum_sc = psb.tile([128, 2], F32, tag="psum_sc")
            nc.tensor.matmul(psum_sc[:, :], o1128_v, pack12[:, :],
                             start=True, stop=True)
            sc_sb = pool.tile([128, 2], F32, tag="sc_sb")
            nc.vector.tensor_copy(sc_sb[:, :], psum_sc[:, :])
            psum_sfbc = psb.tile([128, 32], F32, tag="psum_sfbc")
            nc.tensor.matmul(psum_sfbc[:, :], gr8_v, pool2sf[:, :],
                             start=True, stop=True)
            sfterm = pool.tile([128, 32], F32, tag="sfterm")
            nc.vector.tensor_scalar(sfterm[:, :], psum_sfbc[:, :],
                                    1.0 / 64, sc_sb[:, 0:1], OP.mult, OP.subtract)
            nc.vector.tensor_scalar_mul(sfterm[:, :], sfterm[:, :], sc_sb[:, 1:2])
            flat_f = pool.tile([128, 32], F32, tag="flat_f")
            nc.vector.tensor_scalar_mul(flat_f[:, :], pool2[:, :], 1.0 / 64)
            nc.vector.tensor_tensor(flat_f[:, :], flat_f[:, :], sfterm[:, :], OP.add)
            flat_bf = pool.tile([128, 32], BF16, tag="flat_bf")
            nc.vector.tensor_copy(flat_bf[:, :], flat_f[:, :])

            # MLP
            psum_h = psb.tile([1, 128], F32, tag="psum_h")
            for j in range(32):
                nc.tensor.matmul(psum_h[:, :], flat_bf[:, j:j + 1],
                                 w1_bf[:, j, :], start=(j == 0), stop=(j == 31))
            hr = pool.tile([1, 128], F32, tag="hr")
            nc.scalar.activation(hr[:, :], psum_h[:, :], AF.Relu)
            hw2 = pool.tile([1, 128], F32, tag="hw2")
            nc.vector.tensor_tensor(hw2[:, :], hr[:, :], w2_v, OP.mult)
            dsc = pool.tile([1, 4], F32, tag="dsc")
            nc.vector.tensor_reduce(dsc[:, 0:1], hw2[:, :], AX.X, OP.add)
            # 1/dilation = (1+exp(-x))/MD  (sigmoid via exp: same ACT table)
            nc.scalar.activation(dsc[:, 1:2], dsc[:, 0:1], AF.Exp, scale=-1.0)
            nc.vector.tensor_scalar(dsc[:, 3:4], dsc[:, 1:2], 1.0,
                                    1.0 / float(MD), OP.add, OP.mult)

            # w' = exp(-dist/d)/sum (+1 center)
            psum_i11 = psb.tile([11, 1], F32, tag="psum_i11")
            nc.tensor.matmul(psum_i11[:, :], o111_v, dsc[:, 3:4],
                             start=True, stop=True)
            invd_col = pool.tile([11, 1], F32, tag="invd_col")
            nc.vector.tensor_copy(invd_col[:, :], psum_i11[:, :])
            wexp = pool.tile([11, 11], F32, tag="wexp")
            wrs = pool.tile([11, 1], F32, tag="wrs")
            nc.scalar.activation(wexp[:, :], dnt_v, AF.Exp,
                                 scale=invd_col[:, 0:1],
                                 accum_out=wrs[:, 0:1])
            psum_ws = psb.tile([1, 1], F32, tag="psum_ws")
            nc.tensor.matmul(psum_ws[:, :], o11_v, wrs[:, :],
                             start=True, stop=True)
            wsv = pool.tile([1, 2], F32, tag="wsv")
            nc.vector.reciprocal(wsv[:, 1:2], psum_ws[:, 0:1])
            psum_w11 = psb.tile([11, 1], F32, tag="psum_w11")
            nc.tensor.matmul(psum_w11[:, :], o111_v, wsv[:, 1:2],
                             start=True, stop=True)
            wsi_col = pool.tile([11, 1], F32, tag="wsi_col")
            nc.vector.tensor_copy(wsi_col[:, :], psum_w11[:, :])
            wp_sb = pool.tile([11, 400], F32, tag="wp_sb")
            nc.vector.memset(wp_sb[:, :], 0.0)
            nc.vector.tensor_scalar_mul(wp_sb[:, 250:261], wexp[:, :],
                                        wsi_col[:, 0:1])
            nc.sync.dma_start(out=vdram[:, :], in_=wp_sb[:, :])

            # T matrices via sliding-window DMA + fold
            t_ext = pool.tile([128, 11, 138], F32, tag="t_ext")
            # Load T with all-positive steps (contiguous 552B runs) by
            # storing j REVERSED: t_ext[p, dy, jr] = v[dy, 123 + p + jr]
            # (valid because the kernel rows are symmetric in dx, so the
            # reversed generator equals the original). xe = 132 - jr.
            nc.sync.dma_start(
                out=t_ext[:, :, :],
                in_=AP(tensor=vdram, offset=123,
                       ap=[[1, 128], [400, 11], [1, 138]]))
            # folds in reversed coords: xe=0 target at jr=132 (sources
            # jr 133..137), xe=127 target at jr=5 (sources jr 0..4)
            tl = pool.tile([128, 11, 1], F32, tag="tl")
            th = pool.tile([128, 11, 1], F32, tag="th")
            nc.vector.tensor_reduce(tl[:, :, :], t_ext[:, :, 133:138], AX.X, OP.add)
            nc.vector.tensor_reduce(th[:, :, :], t_ext[:, :, 0:5], AX.X, OP.add)
            nc.vector.tensor_tensor(t_ext[:, :, 132:133], t_ext[:, :, 132:133],
                                    tl[:, :, :], OP.add)
            nc.vector.tensor_tensor(t_ext[:, :, 5:6], t_ext[:, :, 5:6],
                                    th[:, :, :], OP.add)
            # t8[p, r, xout] = T_{dy=5-r}[p, xout] in fp8e4 (reversed dy order
            # so the DoubleRow rhs k-tile step is +64); slot r=11 is zero pad.
            t8 = pool.tile([128, 12, 128], mybir.dt.float8e4, tag="t8")
            nc.vector.memset(t8[:, 11, :], 0.0)
            tea = t_ext[:, :, :]
            nc.vector.tensor_copy(
                t8[:, 0:11, :],
                AP(tensor=tea.tensor, offset=tea.offset + 10 * 138 + 132,
                   ap=[list(tea.ap[0]), [-138, 11], [-1, 128]]))

            # ============== P merge (after scatter chains land) ==============
            # chunked by slot range so conv pass 0 can start after chunk 0
            def pslot_dst(buf, t0, cnt):
                # [128, cnt slots step 2, 64] view into buf starting at slot t0
                a = buf[:, :]
                return AP(tensor=a.tensor, offset=a.offset + t0 * 64,
                          ap=[list(a.ap[0]), [128, cnt], [1, 64]])

            for s0, cnt in [(16, 16), (48, 16), (80, 16), (112, 16)]:
                g0 = (s0 - 16) // 2
                for par in range(2):
                    ev = pbuf[par][:, g0 * 64:(g0 + cnt) * 64].rearrange(
                        "p (a b) -> p a b", b=64)
                    od = pbuf[2 + par][:, g0 * 64:(g0 + cnt) * 64].rearrange(
                        "p (a b) -> p a b", b=64)
                    nc.vector.tensor_tensor(pslot_dst(p8, s0 + par, cnt),
                                            ev, od, OP.add)
                    # exact-P merge on the post-scatter-idle pool engine
                    nc.gpsimd.tensor_tensor(pslot_dst(p_t, s0 + par, cnt),
                                            ev, od, OP.add)

            # ============== Phase C: Toeplitz conv + tail ==============
            # 9 passes x 16 slots with 4 PSUM buffers in flight; ft was
            # prefetched whole during the scatter window so the tail add has
            # no DMA dependency
            psum_stack.close()
            psum_stack = ExitStack()
            psc = psum_stack.enter_context(
                tc.tile_pool(name="psc", bufs=4, space="PSUM"))

            p_t_flat = p_t[:, :]
            p8_flat = p8[:, :]
            DR = mybir.MatmulPerfMode.DoubleRow
            for ps in range(9):
                nslots = 16
                psum_c = psc.tile([128, nslots * 64], F32, tag="psum_c")
                for ch in range(nslots // 8):
                    u0 = 16 * ps + 8 * ch
                    for pi, r in enumerate(range(0, 12, 2)):
                        rhs = AP(tensor=p8_flat.tensor,
                                 offset=p8_flat.offset + (u0 + 3 + r) * 64,
                                 ap=[list(p8_flat.ap[0]), [64, 2], [1, 512]])
                        nc.tensor.matmul(
                            psum_c[:, ch * 512:ch * 512 + 512],
                            t8[:, r:r + 2, :], rhs,
                            start=(pi == 0), stop=False,
                            perf_mode=DR, skip_group_check=True)
                    # exact +P via bf16 identity (center +1 not in t8)
                    nc.tensor.matmul(
                        psum_c[:, ch * 512:ch * 512 + 512], ident_bf,
                        p_t_flat[:, (u0 + 8) * 64:(u0 + 8) * 64 + 512],
                        start=False, stop=True, skip_group_check=True)
                y0 = max(0, 16 * ps - 8)
                y1 = min(128, 16 * ps + 8)
                po = (y0 + 8 - 16 * ps) * 64
                nc.vector.tensor_tensor(
                    ft[:, y0 * 64:y1 * 64],
                    psum_c[:, po:po + (y1 - y0) * 64],
                    ft[:, y0 * 64:y1 * 64], OP.add)
                if ps == 0:
                    pa = psum_c[:, :]
                    tmpf = pool.tile([128, 64], F32, tag="tmpf")
                    nc.vector.tensor_reduce(
                        tmpf[:, :],
                        AP(tensor=pa.tensor, offset=pa.offset + 3 * 64,
                           ap=[list(pa.ap[0]), [1, 64], [64, 5]]),
                        AX.X, OP.add)
                    nc.vector.tensor_tensor(ft[:, 0:64], ft[:, 0:64],
                                            tmpf[:, :], OP.add)
                if ps == 8:
                    pa = psum_c[:, :]
                    tmph = pool.tile([128, 64], F32, tag="tmph")
                    nc.vector.tensor_reduce(
                        tmph[:, :],
                        AP(tensor=pa.tensor, offset=pa.offset + 8 * 64,
                           ap=[list(pa.ap[0]), [1, 64], [64, 5]]),
                        AX.X, OP.add)
                    nc.vector.tensor_tensor(ft[:, 127 * 64:128 * 64],
                                            ft[:, 127 * 64:128 * 64],
                                            tmph[:, :], OP.add)
                nc.sync.dma_start(out=out_T[:, y0 * 64:y1 * 64],
                                  in_=ft[:, y0 * 64:y1 * 64])

            psum_stack.close()

    nc.compile()
    return nc


def build_core_inputs(x, xyz, feature, conv_w, conv_b, gn_gamma, gn_beta,
                      mlp_w1, mlp_w2):
    """Host-side sharding glue: slice batch b per core + layout transforms."""
    import ml_dtypes
    f32 = np.float32
    bf16 = ml_dtypes.bfloat16
    # shared constants
    convw_pl = np.zeros((128, 4, 128), f32)
    sfw_pl = np.zeros((128, 4, 8), f32)
    for h in range(2):
        for g in range(4):
            convw_pl[64 * h:64 * h + 64, g, np.arange(16) * 8 + h * 4 + g] = conv_w.T
            sfw_pl[64 * h:64 * h + 64, g, h * 4 + g] = 1.0
    gmat = (np.arange(128)[:, None] // 32 == np.arange(4)[None, :]).astype(f32)
    ltri_m = (np.arange(128)[:, None] < np.arange(128)[None, :]).astype(f32)
    ident_m = np.eye(128, dtype=f32)
    th_m = np.broadcast_to(128.0 + (np.arange(32)[None, :] % 2),
                           (128, 32)).astype(f32)
    blob1 = np.zeros((128, 968), f32)
    blob1[:, 256] = np.arange(128, dtype=f32)
    blob1[:, 257:289] = th_m
    blob1[:, 960:964] = gmat
    blob1[:, 964] = np.repeat(conv_b, 8)
    blob1[:, 965] = np.repeat(gn_gamma, 8)
    blob1[:, 966] = np.repeat(gn_beta, 8)
    blob1[:, 967] = 1.0
    dxy = np.arange(11) - 5
    blob2 = np.zeros((11, 547), f32)
    blob2[0:4, 0:128] = gmat.T
    blob2[0:8, 128:256] = (np.arange(128)[None, :] % 8
                           == np.arange(8)[:, None]).astype(f32)
    blob2[0, 256:384] = mlp_w2[0]
    blob2[0, 384:512] = 1.0
    blob2[0:11, 512:523] = -np.sqrt(dxy[None, :] ** 2 + dxy[:, None] ** 2)
    blob2[5, 523 + 5] = 1.0
    blob2[0:11, 534] = 1.0
    blob2[0, 535:546] = 1.0
    blob2[0:4, 546] = EPS
    # w1 rearranged: w1n[p=(o,seg), j=(by_l,bx), n] = mlp_w1[n, o*256+(seg*2+by_l)*16+bx]
    o = np.arange(16)[:, None, None, None]
    seg = np.arange(8)[None, :, None, None]
    byl = np.arange(2)[None, None, :, None]
    bx = np.arange(16)[None, None, None, :]
    fl = (o * 256 + (seg * 2 + byl) * 16 + bx).reshape(128, 32)
    w1n = np.ascontiguousarray(mlp_w1.T[fl]).astype(bf16)  # [128, 32, 128]
    # bf16 constant blob: sel8h (wrap one-hots), th, ltri, ones, ident,
    # conv1x1 weights, sf-sum weights
    sel8h = np.zeros((128, 8, 128), f32)
    kk = np.arange(128)
    for gg in range(8):
        sel8h[16 * gg + (kk % 16), gg, kk] = 1.0
    sblob = np.zeros((128, 1858), f32)
    sblob[:, 0:1024] = sel8h.reshape(128, 1024)
    sblob[:, 1024:1056] = th_m * 128.0
    sblob[:, 1056:1184] = ltri_m
    sblob[:, 1184] = 1.0
    sblob[:, 1185:1313] = ident_m
    sblob[:, 1313:1825] = convw_pl.reshape(128, 512)
    sblob[:, 1825:1857] = sfw_pl.reshape(128, 32)
    sblob[:, 1857] = np.arange(128, dtype=f32)
    sblob = np.ascontiguousarray(sblob).astype(bf16)
    shared = dict(blob1=blob1, blob2=blob2, w1n=w1n, sblob=sblob)

    in_maps = []
    for b in range(B):
        fb = np.ascontiguousarray(feature[b].reshape(64, 16384)).astype(bf16)
        m = dict(shared)
        m["x_wrap"] = np.ascontiguousarray(
            x[b].reshape(32, 128, 64).transpose(1, 0, 2)).astype(bf16)
        m["xxy_w2"] = np.ascontiguousarray(np.concatenate(
            [xyz[b, :, 0].reshape(32, 128).T, xyz[b, :, 1].reshape(32, 128).T],
            axis=1)).astype(f32) * np.float32(127.0)
        m["feat_nat2"] = np.concatenate([fb[:, :8192], fb[:, 8192:]], axis=0)
        m["feat_T"] = np.ascontiguousarray(
            feature[b].transpose(2, 1, 0).reshape(128, 8192)).astype(bf16)
        xt = np.zeros((32, 288), f32)
        xt[:, 0:128] = xyz[b, :, 0].reshape(32, 128) * np.float32(127.0)
        xt[:, 128:256] = xyz[b, :, 1].reshape(32, 128) * np.float32(127.0)
        xt[:, 256:288] = np.eye(32, dtype=f32)
        m["xt_blob"] = xt
        in_maps.append(m)
    return in_maps


_NC_CACHE = {}


def kernel(x, xyz, feature, conv_w, conv_b, gn_gamma, gn_beta, mlp_w1, mlp_w2,
           _trace=False):
    from concourse.bass_utils import run_bass_kernel_spmd
    if "nc" not in _NC_CACHE:
        _NC_CACHE["nc"] = build_nc()
    nc = _NC_CACHE["nc"]
    in_maps = build_core_inputs(np.asarray(x), np.asarray(xyz),
                                np.asarray(feature), np.asarray(conv_w),
                                np.asarray(conv_b), np.asarray(gn_gamma),
                                np.asarray(gn_beta), np.asarray(mlp_w1),
                                np.asarray(mlp_w2))
    res = run_bass_kernel_spmd(nc, in_maps, core_ids=list(range(8)),
                               trace=_trace)
    outs = []
    for i in range(B):
        ot = np.asarray(res.results[i]["out_T"]).astype(np.float32)
        outs.append(ot.reshape(128, 128, 64).transpose(2, 1, 0))
    out = np.stack(outs).astype(np.float32)
    if _trace:
        return out, res
    return out



# revision 3
# speedup vs baseline: 1.0184x; 1.0184x over previous
"""Trainium2 Bass kernel for nn_AMM_w_AFDM (scatter_memory).

Strategy (one batch per NeuronCore, 8 cores data-parallel):
  out[b] = feature + P + splat(P, w)  where P = nearest-cell scatter of x.
  The 11x11 splat-with-border-clipping is a bank of banded Toeplitz
  matmuls on TensorE (fp8 DoubleRow); P is built with
  gpsimd.dma_scatter_add (SBUF parity-split CCE-add destination).  The
  exact-dedup (HW races on duplicate indices within a call) runs in
  4-chunk groups pipelined with the scatter calls, with the lin
  broadcast done by stride-0 one-hot matmuls on a transposed floor
  chain (no DRAM roundtrip).  The AFDM conv1x1 runs in 2-PSUM-bank
  blocks during the dedup phase; the dilation scalar chain, the w
  kernel, and the P merges overlap the scatter window and the conv.
  All constants ship pre-cast; the whole kernel uses one ACT table.
"""
import sys
from contextlib import ExitStack
import numpy as np

sys.path.insert(0, "/opt/trn_rl_repo")

import concourse.bacc as bacc  # noqa: E402
import concourse.bass as bass  # noqa: E402
import concourse.mybir as mybir  # noqa: E402
import concourse.tile as tile  # noqa: E402
from concourse.ap import AP  # noqa: E402

MD = 5
EPS = 1e-5
B, N, C, H, W = 8, 4096, 64, 128, 128
F32 = mybir.dt.float32
BF16 = mybir.dt.bfloat16
I16 = mybir.dt.int16
AX = mybir.AxisListType
OP = mybir.AluOpType
AF = mybir.ActivationFunctionType

# P_T slot layout: slot t in [0,160), y = t-16 (zeros outside [0,128)).
PT_SLOTS = 160
PT_OFF = 16
# ext output slots: u in [0,144), ye = u-8.
EXT_SLOTS = 144


def build_nc():
    nc = bacc.Bacc("TRN2", target_bir_lowering=False)

    def din(name, shape, dt=F32):
        return nc.dram_tensor(name, shape, dt, kind="ExternalInput")

    x_wrap = din("x_wrap", [128, 32, 64], BF16)
    xxy_w2 = din("xxy_w2", [128, 64])
    feat_nat2 = din("feat_nat2", [128, 8192], BF16)
    feat_T = din("feat_T", [128, 8192], BF16)
    blob1 = din("blob1", [128, 968])
    blob2 = din("blob2", [11, 547])
    sblob = din("sblob", [128, 1858], BF16)
    xt_blob = din("xt_blob", [32, 288])
    w1n = din("w1n", [128, 32, 128], BF16)

    out_T = nc.dram_tensor("out_T", [128, 8192], BF16, kind="ExternalOutput")
    vdram = nc.dram_tensor("vdram", [11, 400], F32)

    with tile.TileContext(nc) as tc:
        with tc.tile_pool(name="main", bufs=1) as pool, \
             tc.tile_pool(name="scat", bufs=1) as scpool, \
             tc.tile_pool(name="stage", bufs=2) as stpool:
            psum_stack = ExitStack()

            # ============== early input DMAs ==============
            ps_prep = psum_stack.enter_context(
                tc.tile_pool(name="psp", bufs=2, space="PSUM"))
            xt_sb = pool.tile([32, 288], F32, tag="xt_sb")
            nc.sync.dma_start(out=xt_sb[:, :], in_=xt_blob[:, :])
            xxy2 = pool.tile([128, 64], F32, tag="xxy2")
            nc.sync.dma_start(out=xxy2[:, :], in_=xxy_w2[:, :])
            sblob_sb = pool.tile([128, 1858], BF16, tag="sblob_sb")
            nc.sync.dma_start(out=sblob_sb[:, :], in_=sblob[:, :])
            blob1_sb = pool.tile([128, 968], F32, tag="blob1_sb")
            nc.sync.dma_start(out=blob1_sb[:, :], in_=blob1[:, :])
            blob2_sb = pool.tile([11, 547], F32, tag="blob2_sb")
            nc.sync.dma_start(out=blob2_sb[:, :], in_=blob2[:, :])
            x_bf = scpool.tile([128, 32, 64], BF16, tag="x_bf")
            nc.sync.dma_start(out=x_bf[:, :, :], in_=x_wrap[:, :, :])
            fnat_bf = pool.tile([128, 8192], BF16, tag="fnat_bf")
            nc.sync.dma_start(out=fnat_bf[:, :], in_=feat_nat2[:, :])
            w1_bf = pool.tile([128, 32, 128], BF16, tag="w1_bf")

            # ============== Phase S: scatter x -> P ==============
            # Index math in the 128-wrap: lin128[p, t] for point j = t*128+p.
            # (emitted FIRST on the DVE queue so the dedup pipeline starts
            #  immediately; buffer memsets follow on gpsimd/DVE)
            # floor(t) = round(t) - (round(t) > t), round via +/- 2^23;
            # x and y coords packed in one [128,64] tile to halve the serial
            # DVE chain (each small DVE op costs ~1us of fixed overhead)
            # floor(t) = round(t - 0.5) via the 2^23 trick fused into one
            # 2-op tensor_scalar; two layouts: [32,*] (transposed, feeds the
            # dedup broadcast matmuls directly - no PE transpose / DMA fold)
            # and [128,*] (feeds eq-compare scalars and the idx pack).
            # magic constant 1.5*2^23 keeps the sum in the ulp=1 region for
            # all t-0.5 >= -0.5 (plain 2^23 breaks for t<0.25: the sum lands
            # in the 0.5-ulp band just below 2^23).  xyz is pre-scaled by 127
            # host-side so the 2-op tensor_scalar can spend both ops on adds.
            RC = 12582912.0
            rT = pool.tile([32, 256], F32, tag="rT")
            nc.vector.tensor_scalar(rT[:, :], xt_sb[:, 0:256], -0.5,
                                    RC, OP.add, OP.add)
            flT = pool.tile([32, 256], F32, tag="flT")
            nc.vector.tensor_scalar_add(flT[:, :], rT[:, :], -RC)
            linT = pool.tile([32, 128], F32, tag="linT")
            nc.vector.scalar_tensor_tensor(linT[:, :], flT[:, 128:256], 128.0,
                                           flT[:, 0:128], OP.mult, OP.add)
            r128 = pool.tile([128, 64], F32, tag="r128")
            nc.vector.tensor_scalar(r128[:, :], xxy2[:, :], -0.5,
                                    RC, OP.add, OP.add)
            fl = pool.tile([128, 64], F32, tag="fl")
            nc.vector.tensor_scalar_add(fl[:, :], r128[:, :], -RC)
            lin128 = pool.tile([128, 32], F32, tag="lin128")
            nc.vector.scalar_tensor_tensor(lin128[:, :], fl[:, 32:64], 128.0,
                                           fl[:, 0:32], OP.mult, OP.add)

            pcol_col = blob1_sb[:, 256:257]
            th_f = blob1_sb[:, 257:289]
            gm_v = blob1_sb[:, 960:964]
            cb_col = blob1_sb[:, 964:965]
            gg_col = blob1_sb[:, 965:966]
            gb_col = blob1_sb[:, 966:967]
            onesc_f = blob1_sb[:, 967:968]
            gmt_v = blob2_sb[0:4, 0:128]
            gr8_v = blob2_sb[0:8, 128:256]
            w2_v = blob2_sb[0:1, 256:384]
            o1128_v = blob2_sb[0:1, 384:512]
            dnt_v = blob2_sb[0:11, 512:523]
            cm_v = blob2_sb[0:11, 523:534]
            o11_v = blob2_sb[0:11, 534:535]
            o111_v = blob2_sb[0:1, 535:546]
            # bf16 constants shipped pre-cast in sblob (DVE casts of consts
            # were costing ~3us of serial head latency each)
            sba = sblob_sb[:, :]
            _sp = list(sba.ap[0])

            def sb_ap(off, dims):
                return AP(tensor=sba.tensor, offset=sba.offset + off,
                          ap=[_sp] + dims)

            def sel8h_ap(gg):
                return sb_ap(gg * 128, [[1, 128]])

            th_bf_all = sb_ap(1024, [[1, 32]])
            ltri2_ap = sb_ap(1056, [[0, 2], [1, 128]])
            onesc_bf = sb_ap(1184, [[1, 1]])
            ident_bf = sb_ap(1185, [[1, 128]])

            xta = xt_sb[:, :]

            def e32_ap(t0):
                return AP(tensor=xta.tensor, offset=xta.offset + 256 + t0,
                          ap=[list(xta.ap[0]), [0, 128]])

            # post-floor helpers for the idx pack (off the critical chain)
            xcq = pool.tile([128, 32], F32, tag="xcq")
            nc.vector.tensor_scalar(xcq[:, :], fl[:, 0:32], pcol_col,
                                    None, OP.subtract)
            ytr = pool.tile([128, 32], F32, tag="ytr")
            nc.vector.tensor_tensor(ytr[:, :], fl[:, 32:64], th_f, OP.subtract)
            nc.vector.tensor_scalar_mul(ytr[:, :], ytr[:, :], 128.0)

            # scatter destination buffers: memsets on gpsimd, which is idle
            # until the first scatter call anyway — keeps DVE free for dedup
            pb0 = scpool.tile([128, 66 * 64], BF16, tag="pb0")
            pb1 = scpool.tile([128, 66 * 64], BF16, tag="pb1")
            pb2 = scpool.tile([128, 66 * 64], BF16, tag="pb2")
            pb3 = scpool.tile([128, 66 * 64], BF16, tag="pb3")
            pbuf = [pb0, pb1, pb2, pb3]  # [A_even, A_odd, B_even, B_odd]
            # dummy scatter first on the gpsimd queue: forces the Q7 'mlp'
            # library DMA+boot (~10us) at t~4 instead of right before the
            # first real scatter call
            dummy_v = pool.tile([128, 64], BF16, tag="dummy_v")
            nc.vector.memset(dummy_v[:, :], 0.0)
            dummy_i = pool.tile([128, 8], I16, tag="dummy_i")
            nc.vector.memset(dummy_i[:, :], 0)
            # target the never-read trash columns of pb0/pb1: harmless, and
            # the WAW hazard forces the scheduler to run this (and the lib
            # boot it triggers) BEFORE the memsets instead of reordering it
            nc.gpsimd.dma_scatter_add(
                pb0[:, 4096:4160],
                dummy_v[:, :].rearrange("p (a b) -> p a b", b=64),
                dummy_i[:, :], 128, 128, 64,
                parity_reg=0, out_ap_other=pb1[:, 4096:4160],
                sbuf_tokens_per_rank=128)
            for pb in pbuf:
                nc.gpsimd.memset(pb[:, 0:4096], 0.0)
            # P accumulators (bf16 for the exact identity term, fp8 for the
            # DoubleRow Toeplitz conv); border memsets emitted after dedup
            p_t = pool.tile([128, PT_SLOTS * 64], BF16, tag="p_t")
            p8 = pool.tile([128, PT_SLOTS * 64], mybir.dt.float8e4, tag="p8")

            # Per-256-chunk dedup, pipelined in 4-chunk GROUPS so the first
            # scatter call issues as soon as chunks 0-3 are deduped (instead
            # of after a full 8-chunk half).  chunk a = subs (u,v)=(2a,2a+1).
            mqt = [pool.tile([128, 512], BF16, tag=f"mq{i}", name=f"mq{i}")
                   for i in range(2)]
            mlt = [pool.tile([128, 256], BF16, tag=f"ml{i}", name=f"ml{i}")
                   for i in range(2)]
            xmg = [scpool.tile([128, 8, 64], BF16, tag=f"xmg{i}",
                               name=f"xmg{i}") for i in range(4)]
            idxg = [scpool.tile([128, 64], I16, tag=f"idxg{i}",
                                name=f"idxg{i}") for i in range(4)]
            hilot = [pool.tile([128, 16], BF16, tag=f"hilo{i}",
                               name=f"hilo{i}") for i in range(4)]
            psq = psum_stack.enter_context(
                tc.tile_pool(name="psq", bufs=2, space="PSUM"))
            psg = psum_stack.enter_context(
                tc.tile_pool(name="psg", bufs=2, space="PSUM"))
            psA = psum_stack.enter_context(
                tc.tile_pool(name="psA", bufs=2, space="PSUM"))
            r_sb = pool.tile([128, 2048], F32, tag="r_sb")

            for g in range(4):
                hs = slice(8 * g, 8 * g + 8)
                pm = psg.tile([128, 512], F32, tag="pm")
                aux = psg.tile([128, 80], F32, tag="aux")
                cntp = aux[:, 0:8]
                psw = aux[:, 16:80]
                firstp = stpool.tile([128, 8], F32, tag="firstp")
                firstp = firstp[:, :]
                for c in range(4):
                    a = 4 * g + c
                    u, v = 2 * a, 2 * a + 1
                    # bc[q, 0:128|128:256] = lin(p, u|v) bcast across q, via
                    # stride-0 one-hot lhsT on the transposed lin (K=32)
                    bc = psq.tile([128, 256], F32, tag="bc")
                    nc.tensor.matmul(bc[:, 0:128], e32_ap(u), linT[:, :],
                                     start=True, stop=True)
                    nc.tensor.matmul(bc[:, 128:256], e32_ap(v), linT[:, :],
                                     start=True, stop=True)
                    # eq tiles packed as mq = [m_uu | m_uv | m_vu | m_vv]
                    mq = mqt[a % 2]
                    nc.vector.tensor_scalar(mq[:, 0:256], bc[:, 0:256],
                                            lin128[:, u:u + 1], None,
                                            OP.is_equal)
                    nc.vector.tensor_scalar(mq[:, 256:512], bc[:, 0:256],
                                            lin128[:, v:v + 1], None,
                                            OP.is_equal)
                    # ml = [m_uu*L | m_vv*L] in ONE strided op
                    ml = mlt[a % 2]
                    mqa = mq[:, :]
                    nc.vector.tensor_tensor(
                        ml[:, :].rearrange("p (a b) -> p a b", b=128),
                        AP(tensor=mqa.tensor, offset=mqa.offset,
                           ap=[list(mqa.ap[0]), [384, 2], [1, 128]]),
                        ltri2_ap, OP.mult)
                    # merged values
                    nc.tensor.matmul(pm[:, c * 128:c * 128 + 64],
                                     mq[:, 0:128], x_bf[:, u, :],
                                     start=True, stop=False)
                    nc.tensor.matmul(pm[:, c * 128:c * 128 + 64],
                                     mq[:, 256:384], x_bf[:, v, :],
                                     start=False, stop=True)
                    nc.tensor.matmul(pm[:, c * 128 + 64:c * 128 + 128],
                                     mq[:, 384:512], x_bf[:, v, :],
                                     start=True, stop=True)
                    # duplicate-below counts
                    nc.tensor.matmul(cntp[:, 2 * c:2 * c + 1], ml[:, 0:128],
                                     onesc_bf, start=True, stop=True)
                    nc.tensor.matmul(cntp[:, 2 * c + 1:2 * c + 2],
                                     ml[:, 128:256], onesc_bf,
                                     start=True, stop=False)
                    nc.tensor.matmul(cntp[:, 2 * c + 1:2 * c + 2],
                                     mq[:, 128:256], onesc_bf,
                                     start=False, stop=True)
                nc.vector.tensor_scalar(firstp, cntp, 0.5,
                                        None, OP.is_lt)
                nc.scalar.activation(xmg[g][:, :, :], pm[:, :], AF.Copy)
                # idx split into bf16-exact hi=y / lo=x halves (trash cells
                # when not-first), wrapped+replicated by one-hot bf16 matmuls
                hilo = hilot[g]
                nc.vector.tensor_tensor(hilo[:, 0:8], ytr[:, hs],
                                        firstp, OP.mult)
                nc.vector.tensor_tensor(
                    hilo[:, 0:8], hilo[:, 0:8],
                    AP(tensor=sba.tensor, offset=sba.offset + 1024 + 8 * g,
                       ap=[_sp, [1, 8]]), OP.add)
                nc.vector.tensor_tensor(hilo[:, 8:16], xcq[:, hs],
                                        firstp, OP.mult)
                nc.vector.tensor_tensor(hilo[:, 8:16], hilo[:, 8:16],
                                        sb_ap(1857, [[0, 8]]), OP.add)
                for gg in range(8):
                    pw_ap = AP(tensor=psw.tensor, offset=psw.offset + gg * 8,
                               ap=[list(psw.ap[0]), [1, 8]])
                    nc.tensor.matmul(pw_ap, sel8h_ap(gg), hilo[:, 8:16],
                                     start=True, stop=False)
                    nc.tensor.matmul(pw_ap, sel8h_ap(gg), hilo[:, 0:8],
                                     start=False, stop=True)
                ida = idxg[g][:, :]
                nc.vector.tensor_copy(
                    AP(tensor=ida.tensor, offset=ida.offset,
                       ap=[list(ida.ap[0]), [16, 4], [8, 2], [1, 8]]),
                    AP(tensor=psw.tensor, offset=psw.offset,
                       ap=[list(psw.ap[0]), [2, 4], [1, 2], [8, 8]]))
                for c in range(4):
                    a = 4 * g + c
                    ch = a % 2
                    nc.gpsimd.dma_scatter_add(
                        pbuf[2 * ch][:, :], xmg[g][:, 2 * c:2 * c + 2, :],
                        idxg[g][:, c * 16:(c + 1) * 16],
                        256, 256, 64,
                        parity_reg=0, out_ap_other=pbuf[2 * ch + 1][:, :],
                        sbuf_tokens_per_rank=128,
                    )

            # AFDM conv1x1 in 2-bank blocks DURING the dedup/scatter phase
            # (fits beside the dedup psum pools), spilled to r_sb so the GN
            # stats chain starts ~20us earlier
            for blk in range(4):
                psr = psA.tile([128, 512], F32, tag="psr")
                for g in range(4):
                    nc.tensor.matmul(psr[:, :],
                                     sb_ap(1313 + 128 * g, [[1, 128]]),
                                     fnat_bf[:, (g * 4 + blk) * 512:
                                             (g * 4 + blk) * 512 + 512],
                                     start=(g == 0), stop=(g == 3))
                nc.scalar.activation(r_sb[:, blk * 512:(blk + 1) * 512],
                                     psr[:, :], AF.Copy)

            # big input DMAs deferred here: they land during the scatter
            # window instead of clogging SBUF write ports during the dedup
            # index math at the head
            nc.sync.dma_start(out=w1_bf[:, :, :], in_=w1n[:, :, :])
            ft = pool.tile([128, 8192], BF16, tag="ft")
            nc.sync.dma_start(out=ft[:, :], in_=feat_T[:, :])

            # P border slots (small; DVE reaches these right after dedup)
            nc.vector.memset(p_t[:, 0:PT_OFF * 64], 0.0)
            nc.vector.memset(p_t[:, (PT_OFF + 128) * 64:], 0.0)
            nc.vector.memset(p8[:, 0:PT_OFF * 64], 0.0)
            nc.vector.memset(p8[:, (PT_OFF + 128) * 64:], 0.0)

            psum_stack.close()
            psum_stack = ExitStack()
            psa = psum_stack.enter_context(
                tc.tile_pool(name="psa", bufs=1, space="PSUM"))

            # ============== Phase A: AFDM dilation ==============
            # (emitted before the P merge so its DVE/ACT/PE ops run during the
            #  scatter window)
            # sf sums (conv_b is zeros per the problem spec, so r_sb needs
            # no bias pass; it was spilled block-wise during the dedup phase)
            psum_sf = psa.tile([8, 2048], F32, tag="psum_sf")
            for wnd in range(16):
                g = wnd // 4
                st_flag = wnd < 4
                sp_flag = wnd >= 12
                rhs = fnat_bf[:, wnd * 512:(wnd + 1) * 512]
                nc.tensor.matmul(psum_sf[:, (wnd % 4) * 512:(wnd % 4) * 512 + 512],
                                 sb_ap(1825 + 8 * g, [[1, 8]]), rhs,
                                 start=st_flag, stop=sp_flag)
            # sf: min/max + pool
            sfmm = pool.tile([8, 2], F32, tag="sfmm")
            nc.vector.tensor_reduce(sfmm[:, 0:1], psum_sf[:, :], AX.X, OP.min)
            nc.vector.tensor_reduce(sfmm[:, 1:2], psum_sf[:, :], AX.X, OP.max)
            pool1sf = pool.tile([8, 256], F32, tag="pool1sf")
            nc.vector.tensor_reduce(pool1sf[:, :],
                                    psum_sf[:, :].rearrange("p (a b) -> p a b", b=8),
                                    AX.X, OP.add)
            pool2sf = pool.tile([8, 32], F32, tag="pool2sf")
            p1s = pool1sf[:, :]
            nc.vector.tensor_reduce(
                pool2sf[:, :].rearrange("p (a b) -> p a b", b=16),
                AP(tensor=p1s.tensor, offset=p1s.offset,
                   ap=[list(p1s.ap[0]), [128, 2], [1, 16], [16, 8]]),
                AX.X, OP.add)

            # close psa (psum_r / psum_sf fully consumed), open psb
            psum_stack.close()
            psum_stack = ExitStack()
            psb = psum_stack.enter_context(
                tc.tile_pool(name="psb", bufs=1, space="PSUM"))
            # stats: s1 = sum r, s2 = sum r^2 (ACT Square w/ accum)
            s_col = pool.tile([128, 2], F32, tag="s_col")
            nc.vector.tensor_reduce(s_col[:, 0:1], r_sb[:, :], AX.X, OP.add)
            rr_sb = pool.tile([128, 2048], BF16, tag="rr_sb")
            nc.scalar.activation(rr_sb[:, :], r_sb[:, :], AF.Square,
                                 accum_out=s_col[:, 1:2])
            psum_st = psb.tile([4, 2], F32, tag="psum_st")
            nc.tensor.matmul(psum_st[:, :], gm_v, s_col[:, :],
                             start=True, stop=True)
            # mu, rstd
            stt = pool.tile([4, 6], F32, tag="stt")
            inv_cnt = 1.0 / (4 * H * W)
            nc.vector.tensor_scalar_mul(stt[:, 0:1], psum_st[:, 0:1], inv_cnt)
            nc.vector.tensor_tensor(stt[:, 2:3], stt[:, 0:1], stt[:, 0:1], OP.mult)
            nc.vector.scalar_tensor_tensor(stt[:, 3:4], psum_st[:, 1:2],
                                           inv_cnt, stt[:, 2:3],
                                           OP.mult, OP.subtract)
            # rstd = exp(-0.5*ln(var+eps)): keeps every ACT op in the single
            # natural_log_exp_and_others table (no mid-kernel table swaps)
            nc.scalar.activation(stt[:, 4:5], stt[:, 3:4], AF.Ln,
                                 bias=blob2_sb[0:4, 546:547])
            nc.scalar.activation(stt[:, 1:2], stt[:, 4:5], AF.Exp, scale=-0.5)
            sta = stt[:, :]
            psum_bc = psb.tile([128, 2], F32, tag="psum_bc")
            nc.tensor.matmul(psum_bc[:, :], gmt_v,
                             AP(tensor=sta.tensor, offset=sta.offset,
                                ap=[list(sta.ap[0]), [1, 2]]),
                             start=True, stop=True)
            a_col = pool.tile([128, 1], F32, tag="a_col")
            b_col = pool.tile([128, 1], F32, tag="b_col")
            nc.vector.tensor_tensor(a_col[:, :], psum_bc[:, 1:2], gg_col[:, :], OP.mult)
            nc.vector.tensor_tensor(b_col[:, :], psum_bc[:, 0:1], a_col[:, :], OP.mult)
            nc.vector.tensor_tensor(b_col[:, :], gb_col[:, :], b_col[:, :], OP.subtract)
            # rr = relu(r*a + b)
            nc.scalar.activation(rr_sb[:, :], r_sb[:, :], AF.Relu,
                                 bias=b_col[:, 0:1], scale=a_col[:, 0:1])
            # pool 8x8
            pool1 = pool.tile([128, 256], F32, tag="pool1")
            nc.vector.tensor_reduce(pool1[:, :],
                                    rr_sb[:, :].rearrange("p (a b) -> p a b", b=8),
                                    AX.X, OP.add)
            pool2 = pool.tile([128, 32], F32, tag="pool2")
            p1a = pool1[:, :]
            nc.vector.tensor_reduce(
                pool2[:, :].rearrange("p (a b) -> p a b", b=16),
                AP(tensor=p1a.tensor, offset=p1a.offset,
                   ap=[list(p1a.ap[0]), [128, 2], [1, 16], [16, 8]]),
                AX.X, OP.add)

            mm2t = pool.tile([1, 16], F32, tag="mm2t")
            nc.sync.dma_start(out=mm2t[:, :], in_=sfmm[:, :])
            gmn = pool.tile([1, 4], F32, tag="gmn")
            mma = mm2t[:, :]
            nc.vector.tensor_reduce(
                gmn[:, 0:1],
                AP(tensor=mma.tensor, offset=mma.offset, ap=[list(mma.ap[0]), [2, 8]]),
                AX.X, OP.min)
            nc.vector.tensor_reduce(
                gmn[:, 1:2],
                AP(tensor=mma.tensor, offset=mma.offset + 1,
                   ap=[list(mma.ap[0]), [2, 8]]),
                AX.X, OP.max)
            nc.vector.tensor_tensor(gmn[:, 2:3], gmn[:, 1:2], gmn[:, 0:1], OP.subtract)
            nc.vector.tensor_scalar_add(gmn[:, 2:3], gmn[:, 2:3], EPS)
            nc.vector.reciprocal(gmn[:, 3:4], gmn[:, 2:3])
            pack12 = pool.tile([1, 2], F32, tag="pack12")
            nc.vector.tensor_copy(pack12[:, 0:1], gmn[:, 0:1])
            nc.vector.tensor_copy(pack12[:, 1:2], gmn[:, 3:4])
            psum_sc = psb.tile([128, 2], F32, tag="psum_sc")
            nc.tensor.matmul(psum_sc[:, :], o1128_v, pack12[:, :],
                             start=True, stop=True)
            sc_sb = pool.tile([128, 2], F32, tag="sc_sb")
            nc.vector.tensor_copy(sc_sb[:, :], psum_sc[:, :])
            psum_sfbc = psb.tile([128, 32], F32, tag="psum_sfbc")
            nc.tensor.matmul(psum_sfbc[:, :], gr8_v, pool2sf[:, :],
                             start=True, stop=True)
            sfterm = pool.tile([128, 32], F32, tag="sfterm")
            nc.vector.tensor_scalar(sfterm[:, :], psum_sfbc[:, :],
                                    1.0 / 64, sc_sb[:, 0:1], OP.mult, OP.subtract)
            nc.vector.tensor_scalar_mul(sfterm[:, :], sfterm[:, :], sc_sb[:, 1:2])
            flat_f = pool.tile([128, 32], F32, tag="flat_f")
            nc.vector.tensor_scalar_mul(flat_f[:, :], pool2[:, :], 1.0 / 64)
            nc.vector.tensor_tensor(flat_f[:, :], flat_f[:, :], sfterm[:, :], OP.add)
            flat_bf = pool.tile([128, 32], BF16, tag="flat_bf")
            nc.vector.tensor_copy(flat_bf[:, :], flat_f[:, :])

            # MLP
            psum_h = psb.tile([1, 128], F32, tag="psum_h")
            for j in range(32):
                nc.tensor.matmul(psum_h[:, :], flat_bf[:, j:j + 1],
                                 w1_bf[:, j, :], start=(j == 0), stop=(j == 31))
            hr = pool.tile([1, 128], F32, tag="hr")
            nc.scalar.activation(hr[:, :], psum_h[:, :], AF.Relu)
            hw2 = pool.tile([1, 128], F32, tag="hw2")
            nc.vector.tensor_tensor(hw2[:, :], hr[:, :], w2_v, OP.mult)
            dsc = pool.tile([1, 4], F32, tag="dsc")
            nc.vector.tensor_reduce(dsc[:, 0:1], hw2[:, :], AX.X, OP.add)
            # 1/dilation = (1+exp(-x))/MD  (sigmoid via exp: same ACT table)
            nc.scalar.activation(dsc[:, 1:2], dsc[:, 0:1], AF.Exp, scale=-1.0)
            nc.vector.tensor_scalar(dsc[:, 3:4], dsc[:, 1:2], 1.0,
                                    1.0 / float(MD), OP.add, OP.mult)

            # w' = exp(-dist/d)/sum (+1 center)
            psum_i11 = psb.tile([11, 1], F32, tag="psum_i11")
            nc.tensor.matmul(psum_i11[:, :], o111_v, dsc[:, 3:4],
                             start=True, stop=True)
            invd_col = pool.tile([11, 1], F32, tag="invd_col")
            nc.vector.tensor_copy(invd_col[:, :], psum_i11[:, :])
            wexp = pool.tile([11, 11], F32, tag="wexp")
            wrs = pool.tile([11, 1], F32, tag="wrs")
            nc.scalar.activation(wexp[:, :], dnt_v, AF.Exp,
                                 scale=invd_col[:, 0:1],
                                 accum_out=wrs[:, 0:1])
            psum_ws = psb.tile([1, 1], F32, tag="psum_ws")
            nc.tensor.matmul(psum_ws[:, :], o11_v, wrs[:, :],
                             start=True, stop=True)
            wsv = pool.tile([1, 2], F32, tag="wsv")
            nc.vector.reciprocal(wsv[:, 1:2], psum_ws[:, 0:1])
            psum_w11 = psb.tile([11, 1], F32, tag="psum_w11")
            nc.tensor.matmul(psum_w11[:, :], o111_v, wsv[:, 1:2],
                             start=True, stop=True)
            wsi_col = pool.tile([11, 1], F32, tag="wsi_col")
            nc.vector.tensor_copy(wsi_col[:, :], psum_w11[:, :])
            wp_sb = pool.tile([11, 400], F32, tag="wp_sb")
            nc.vector.memset(wp_sb[:, :], 0.0)
            nc.vector.tensor_scalar_mul(wp_sb[:, 250:261], wexp[:, :],
                                        wsi_col[:, 0:1])
            nc.sync.dma_start(out=vdram[:, :], in_=wp_sb[:, :])

            # T matrices via sliding-window DMA + fold
            t_ext = pool.tile([128, 11, 138], F32, tag="t_ext")
            # Load T with all-positive steps (contiguous 552B runs) by
            # storing j REVERSED: t_ext[p, dy, jr] = v[dy, 123 + p + jr]
            # (valid because the kernel rows are symmetric in dx, so the
            # reversed generator equals the original). xe = 132 - jr.
            nc.sync.dma_start(
                out=t_ext[:, :, :],
                in_=AP(tensor=vdram, offset=123,
                       ap=[[1, 128], [400, 11], [1, 138]]))
            # folds in reversed coords: xe=0 target at jr=132 (sources
            # jr 133..137), xe=127 target at jr=5 (sources jr 0..4)
            tl = pool.tile([128, 11, 1], F32, tag="tl")
            th = pool.tile([128, 11, 1], F32, tag="th")
            nc.vector.tensor_reduce(tl[:, :, :], t_ext[:, :, 133:138], AX.X, OP.add)
            nc.vector.tensor_reduce(th[:, :, :], t_ext[:, :, 0:5], AX.X, OP.add)
            nc.vector.tensor_tensor(t_ext[:, :, 132:133], t_ext[:, :, 132:133],
                                    tl[:, :, :], OP.add)
            nc.vector.tensor_tensor(t_ext[:, :, 5:6], t_ext[:, :, 5:6],
                                    th[:, :, :], OP.add)
            # t8[p, r, xout] = T_{dy=5-r}[p, xout] in fp8e4 (reversed dy order
            # so the DoubleRow rhs k-tile step is +64); slot r=11 is zero pad.
            t8 = pool.tile([128, 12, 128], mybir.dt.float8e4, tag="t8")
            nc.vector.memset(t8[:, 11, :], 0.0)
            tea = t_ext[:, :, :]
            nc.vector.tensor_copy(
                t8[:, 0:11, :],
                AP(tensor=tea.tensor, offset=tea.offset + 10 * 138 + 132,
                   ap=[list(tea.ap[0]), [-138, 11], [-1, 128]]))

            # ============== P merge (after scatter chains land) ==============
            # chunked by slot range so conv pass 0 can start after chunk 0
            def pslot_dst(buf, t0, cnt):
                # [128, cnt slots step 2, 64] view into buf starting at slot t0
                a = buf[:, :]
                return AP(tensor=a.tensor, offset=a.offset + t0 * 64,
                          ap=[list(a.ap[0]), [128, cnt], [1, 64]])

            for s0, cnt in [(16, 16), (48, 16), (80, 16), (112, 16)]:
                g0 = (s0 - 16) // 2
                for par in range(2):
                    ev = pbuf[par][:, g0 * 64:(g0 + cnt) * 64].rearrange(
                        "p (a b) -> p a b", b=64)
                    od = pbuf[2 + par][:, g0 * 64:(g0 + cnt) * 64].rearrange(
                        "p (a b) -> p a b", b=64)
                    nc.vector.tensor_tensor(pslot_dst(p8, s0 + par, cnt),
                                            ev, od, OP.add)
                    # exact-P merge on the post-scatter-idle pool engine
                    nc.gpsimd.tensor_tensor(pslot_dst(p_t, s0 + par, cnt),
                                            ev, od, OP.add)

            # ============== Phase C: Toeplitz conv + tail ==============
            # 9 passes x 16 slots with 4 PSUM buffers in flight; ft was
            # prefetched whole during the scatter window so the tail add has
            # no DMA dependency
            psum_stack.close()
            psum_stack = ExitStack()
            psc = psum_stack.enter_context(
                tc.tile_pool(name="psc", bufs=4, space="PSUM"))

            p_t_flat = p_t[:, :]
            p8_flat = p8[:, :]
            DR = mybir.MatmulPerfMode.DoubleRow
            for ps in range(9):
                nslots = 16
                psum_c = psc.tile([128, nslots * 64], F32, tag="psum_c")
                for ch in range(nslots // 8):
                    u0 = 16 * ps + 8 * ch
                    for pi, r in enumerate(range(0, 12, 2)):
                        rhs = AP(tensor=p8_flat.tensor,
                                 offset=p8_flat.offset + (u0 + 3 + r) * 64,
                                 ap=[list(p8_flat.ap[0]), [64, 2], [1, 512]])
                        nc.tensor.matmul(
                            psum_c[:, ch * 512:ch * 512 + 512],
                            t8[:, r:r + 2, :], rhs,
                            start=(pi == 0), stop=(pi == 5),
                            perf_mode=DR, skip_group_check=True)
                y0 = max(0, 16 * ps - 8)
                y1 = min(128, 16 * ps + 8)
                po = (y0 + 8 - 16 * ps) * 64
                nc.vector.tensor_tensor(
                    ft[:, y0 * 64:y1 * 64],
                    psum_c[:, po:po + (y1 - y0) * 64],
                    ft[:, y0 * 64:y1 * 64], OP.add)
                nc.vector.tensor_tensor(
                    ft[:, y0 * 64:y1 * 64],
                    ft[:, y0 * 64:y1 * 64],
                    p_t_flat[:, (y0 + 16) * 64:(y1 + 16) * 64], OP.add)
                if ps == 0:
                    pa = psum_c[:, :]
                    tmpf = pool.tile([128, 64], F32, tag="tmpf")
                    nc.vector.tensor_reduce(
                        tmpf[:, :],
                        AP(tensor=pa.tensor, offset=pa.offset + 3 * 64,
                           ap=[list(pa.ap[0]), [1, 64], [64, 5]]),
                        AX.X, OP.add)
                    nc.vector.tensor_tensor(ft[:, 0:64], ft[:, 0:64],
                                            tmpf[:, :], OP.add)
                if ps == 8:
                    pa = psum_c[:, :]
                    tmph = pool.tile([128, 64], F32, tag="tmph")
                    nc.vector.tensor_reduce(
                        tmph[:, :],
                        AP(tensor=pa.tensor, offset=pa.offset + 8 * 64,
                           ap=[list(pa.ap[0]), [1, 64], [64, 5]]),
                        AX.X, OP.add)
                    nc.vector.tensor_tensor(ft[:, 127 * 64:128 * 64],
                                            ft[:, 127 * 64:128 * 64],
                                            tmph[:, :], OP.add)
                nc.sync.dma_start(out=out_T[:, y0 * 64:y1 * 64],
                                  in_=ft[:, y0 * 64:y1 * 64])

            psum_stack.close()

    nc.compile()
    return nc


def build_core_inputs(x, xyz, feature, conv_w, conv_b, gn_gamma, gn_beta,
                      mlp_w1, mlp_w2):
    """Host-side sharding glue: slice batch b per core + layout transforms."""
    import ml_dtypes
    f32 = np.float32
    bf16 = ml_dtypes.bfloat16
    # shared constants
    convw_pl = np.zeros((128, 4, 128), f32)
    sfw_pl = np.zeros((128, 4, 8), f32)
    for h in range(2):
        for g in range(4):
            convw_pl[64 * h:64 * h + 64, g, np.arange(16) * 8 + h * 4 + g] = conv_w.T
            sfw_pl[64 * h:64 * h + 64, g, h * 4 + g] = 1.0
    gmat = (np.arange(128)[:, None] // 32 == np.arange(4)[None, :]).astype(f32)
    ltri_m = (np.arange(128)[:, None] < np.arange(128)[None, :]).astype(f32)
    ident_m = np.eye(128, dtype=f32)
    th_m = np.broadcast_to(128.0 + (np.arange(32)[None, :] % 2),
                           (128, 32)).astype(f32)
    blob1 = np.zeros((128, 968), f32)
    blob1[:, 256] = np.arange(128, dtype=f32)
    blob1[:, 257:289] = th_m
    blob1[:, 960:964] = gmat
    blob1[:, 964] = np.repeat(conv_b, 8)
    blob1[:, 965] = np.repeat(gn_gamma, 8)
    blob1[:, 966] = np.repeat(gn_beta, 8)
    blob1[:, 967] = 1.0
    dxy = np.arange(11) - 5
    blob2 = np.zeros((11, 547), f32)
    blob2[0:4, 0:128] = gmat.T
    blob2[0:8, 128:256] = (np.arange(128)[None, :] % 8
                           == np.arange(8)[:, None]).astype(f32)
    blob2[0, 256:384] = mlp_w2[0]
    blob2[0, 384:512] = 1.0
    blob2[0:11, 512:523] = -np.sqrt(dxy[None, :] ** 2 + dxy[:, None] ** 2)
    blob2[5, 523 + 5] = 1.0
    blob2[0:11, 534] = 1.0
    blob2[0, 535:546] = 1.0
    blob2[0:4, 546] = EPS
    # w1 rearranged: w1n[p=(o,seg), j=(by_l,bx), n] = mlp_w1[n, o*256+(seg*2+by_l)*16+bx]
    o = np.arange(16)[:, None, None, None]
    seg = np.arange(8)[None, :, None, None]
    byl = np.arange(2)[None, None, :, None]
    bx = np.arange(16)[None, None, None, :]
    fl = (o * 256 + (seg * 2 + byl) * 16 + bx).reshape(128, 32)
    w1n = np.ascontiguousarray(mlp_w1.T[fl]).astype(bf16)  # [128, 32, 128]
    # bf16 constant blob: sel8h (wrap one-hots), th, ltri, ones, ident,
    # conv1x1 weights, sf-sum weights
    sel8h = np.zeros((128, 8, 128), f32)
    kk = np.arange(128)
    for gg in range(8):
        sel8h[16 * gg + (kk % 16), gg, kk] = 1.0
    sblob = np.zeros((128, 1858), f32)
    sblob[:, 0:1024] = sel8h.reshape(128, 1024)
    sblob[:, 1024:1056] = th_m * 128.0
    sblob[:, 1056:1184] = ltri_m
    sblob[:, 1184] = 1.0
    sblob[:, 1185:1313] = ident_m
    sblob[:, 1313:1825] = convw_pl.reshape(128, 512)
    sblob[:, 1825:1857] = sfw_pl.reshape(128, 32)
    sblob[:, 1857] = np.arange(128, dtype=f32)
    sblob = np.ascontiguousarray(sblob).astype(bf16)
    shared = dict(blob1=blob1, blob2=blob2, w1n=w1n, sblob=sblob)

    in_maps = []
    for b in range(B):
        fb = np.ascontiguousarray(feature[b].reshape(64, 16384)).astype(bf16)
        m = dict(shared)
        m["x_wrap"] = np.ascontiguousarray(
            x[b].reshape(32, 128, 64).transpose(1, 0, 2)).astype(bf16)
        m["xxy_w2"] = np.ascontiguousarray(np.concatenate(
            [xyz[b, :, 0].reshape(32, 128).T, xyz[b, :, 1].reshape(32, 128).T],
            axis=1)).astype(f32) * np.float32(127.0)
        m["feat_nat2"] = np.concatenate([fb[:, :8192], fb[:, 8192:]], axis=0)
        m["feat_T"] = np.ascontiguousarray(
            feature[b].transpose(2, 1, 0).reshape(128, 8192)).astype(bf16)
        xt = np.zeros((32, 288), f32)
        xt[:, 0:128] = xyz[b, :, 0].reshape(32, 128) * np.float32(127.0)
        xt[:, 128:256] = xyz[b, :, 1].reshape(32, 128) * np.float32(127.0)
        xt[:, 256:288] = np.eye(32, dtype=f32)
        m["xt_blob"] = xt
        in_maps.append(m)
    return in_maps


_NC_CACHE = {}


def kernel(x, xyz, feature, conv_w, conv_b, gn_gamma, gn_beta, mlp_w1, mlp_w2,
           _trace=False):
    from concourse.bass_utils import run_bass_kernel_spmd
    if "nc" not in _NC_CACHE:
        _NC_CACHE["nc"] = build_nc()
    nc = _NC_CACHE["nc"]
    in_maps = build_core_inputs(np.asarray(x), np.asarray(xyz),
                                np.asarray(feature), np.asarray(conv_w),
                                np.asarray(conv_b), np.asarray(gn_gamma),
                                np.asarray(gn_beta), np.asarray(mlp_w1),
                                np.asarray(mlp_w2))
    res = run_bass_kernel_spmd(nc, in_maps, core_ids=list(range(8)),
                               trace=_trace)
    outs = []
    for i in range(B):
        ot = np.asarray(res.results[i]["out_T"]).astype(np.float32)
        outs.append(ot.reshape(128, 128, 64).transpose(2, 1, 0))
    out = np.stack(outs).astype(np.float32)
    if _trace:
        return out, res
    return out

